# revision 12
# baseline (speedup 1.0000x reference)
"""Trainium2 Bass kernel for the MDAM-like module.

Math: the reference's coordinate-attention branch is dead code — its only
consumer is mean(xn) over (h, w), which is identically gn_b because
mean(gated - mu) == 0.  So x_att = softmax(gn_b), a constant per channel.
GAP(conv3x3(x, SAME) + b3) collapses to a 9-term expression over per-channel
total/border/corner sums:
    T[c, di, dj] = S[c] - R(di)[c] - C(dj)[c] + P(di, dj)[c]
    y_gap[o] = b3[o] + (1/HW) * sum_{c,di,dj} w3[o, c, di, dj] * T[c, di, dj]
with R(0)=last-row sum, R(2)=first-row sum, C(0)=last-col, C(2)=first-col,
and P the doubly-excluded corner pixel.  The rest is a per-group 16-dim MLP:
    weights = sigmoid(softmax(gn_b) + sigmoid(fc2 @ relu(fc1 @ y_gap)))
    out = x * weights
so the kernel is one streaming pass: per-channel reductions + tiny matmuls +
one broadcast multiply.  Data-parallel over the batch dim across 8 cores.

Layout per core: x[b] viewed as [512 channels, 128*128].  4 supertiles of
128 channels (= 8 groups x 16 ch) on the 128 SBUF partitions; the group
structure maps to block-diagonal 128x128 matmul weights (prepared host-side).
"""

import numpy as np

H = 128
W = 128
HW = H * W
CG = 16          # channels per group
GPS = 8          # groups per supertile (128 partitions / 16)
NSUP = 4         # supertiles per core (512 channels / 128)
NCORES = 8
CH = 512         # channels per core

_BASS_CACHE = {}


def _build_bass():
    import concourse.bass as bass
    import concourse.tile as tile
    from concourse import mybir

    # The kernel-tail Drain waits on every active proc (11 sems) but this
    # walrus codegen caps sem waits per instruction. Split it: one partial
    # drain per proc (1 wait each), then the original full drain's waits are
    # elided by add_sem_waits' per-engine observed-clock check.
    from concourse.vector_clock import ScopedClock, VectorClock

    if not getattr(tile.TileContext, "_drain_split_patched", False):
        _orig_drain = tile.TileContext._drain_and_barrier

        def _split_drain(self, tick_clock, wait_clock):
            gc = tick_clock.global_clock
            vec = list(gc)
            for proc, tick in enumerate(vec):
                if tick <= 0:
                    continue
                chunk = [0] * len(vec)
                chunk[proc] = tick
                d = self.nc.sync.drain()
                wait_clock.add_sem_waits(
                    d.ins, ScopedClock({None: VectorClock(chunk)})
                )
            # The partial drains above cover the whole clock (SP executes them
            # in order); hand the original a zeroed clock so its drain carries
            # no waits and just anchors the barrier/cleanup tail.
            import types

            shim = types.SimpleNamespace(global_clock=VectorClock([0] * len(vec)))
            _orig_drain(self, shim, wait_clock)

        tile.TileContext._drain_and_barrier = _split_drain
        tile.TileContext._drain_split_patched = True

    nc = bass.Bass(trn_type="TRN2")
    fp32 = mybir.dt.float32
    AF = mybir.ActivationFunctionType
    ALU = mybir.AluOpType
    AX = mybir.AxisListType

    x = nc.dram_tensor("x", [CH, HW], fp32, kind="ExternalInput")
    # w3bd [128,1152] ++ fcbd [128,256] ++ bvec [128,2] in one tensor so the
    # const load is a single DMA (HWDGE has exactly 8 sem lanes; the 4 loads +
    # 4 stores must not wrap lanes or every wrapped DMA gains a same-lane
    # ordering wait on top of its real dep, exceeding the 1-wait HW limit).
    params = nc.dram_tensor("params", [128, 1410], fp32, kind="ExternalInput")
    out = nc.dram_tensor("out", [CH, HW], fp32, kind="ExternalOutput")

    with tile.TileContext(nc) as tc:
        with (
            tc.tile_pool(name="data", bufs=2) as data_pool,
            tc.tile_pool(name="consts", bufs=1) as consts,
            tc.tile_pool(name="small", bufs=4) as small,
            tc.tile_pool(name="psum", bufs=2, space="PSUM") as psum_pool,
        ):
            # PE Matmult carries at most ONE cross-engine sem wait, so every
            # matmul operand must be produced on DVE: DMA the consts to a raw
            # tile (via SWDGE to keep the 8 HWDGE lanes for the big DMAs),
            # then DVE-copy to the tile PE/DVE/ACT actually read.
            praw = consts.tile([128, 1410], fp32)
            nc.gpsimd.dma_start(out=praw, in_=params[:, :])
            pcst = consts.tile([128, 1410], fp32)
            nc.vector.tensor_copy(out=pcst, in_=praw)
            w3s = pcst[:, 0:1152]
            fcs = pcst[:, 1152:1408]
            bv = pcst[:, 1408:1410]

            for t in range(NSUP):
                X = data_pool.tile([128, HW], fp32, tag="X")
                nc.sync.dma_start(out=X, in_=x[128 * t : 128 * (t + 1), :])
                X3 = X.rearrange("p (h w) -> p h w", w=W)

                # Row sums [128, H]; S = total; first/last col sums.
                RS = small.tile([128, H], fp32, tag="RS")
                nc.vector.reduce_sum(out=RS, in_=X3, axis=AX.X)
                S = small.tile([128, 1], fp32, tag="S")
                nc.vector.reduce_sum(out=S, in_=RS, axis=AX.X)
                c0 = small.tile([128, 1], fp32, tag="c0")
                nc.vector.reduce_sum(out=c0, in_=X3[:, :, 0:1], axis=AX.XY)
                cL = small.tile([128, 1], fp32, tag="cL")
                nc.vector.reduce_sum(out=cL, in_=X3[:, :, W - 1 : W], axis=AX.XY)

                r0 = RS[:, 0:1]
                rL = RS[:, H - 1 : H]

                # A[di] = S - R(di):  A0 = S - rL, A1 = S, A2 = S - r0
                A = small.tile([128, 3], fp32, tag="A")
                nc.vector.tensor_sub(out=A[:, 0:1], in0=S, in1=rL)
                nc.vector.tensor_copy(out=A[:, 1:2], in_=S)
                nc.vector.tensor_sub(out=A[:, 2:3], in0=S, in1=r0)

                corner = {
                    (0, 0): X[:, HW - 1 : HW],          # pixel (127,127)
                    (0, 2): X[:, HW - W : HW - W + 1],  # pixel (127,0)
                    (2, 0): X[:, W - 1 : W],            # pixel (0,127)
                    (2, 2): X[:, 0:1],                  # pixel (0,0)
                }
                T = small.tile([128, 9], fp32, tag="T")
                for di in range(3):
                    for dj in range(3):
                        k = 3 * di + dj
                        if dj == 1:
                            nc.vector.tensor_copy(out=T[:, k : k + 1], in_=A[:, di : di + 1])
                        else:
                            cexc = cL if dj == 0 else c0
                            if di == 1:
                                nc.vector.tensor_sub(
                                    out=T[:, k : k + 1], in0=A[:, 1:2], in1=cexc
                                )
                            else:
                                nc.vector.tensor_scalar(
                                    out=T[:, k : k + 1],
                                    in0=A[:, di : di + 1],
                                    scalar1=cexc,
                                    scalar2=corner[(di, dj)],
                                    op0=ALU.subtract,
                                    op1=ALU.add,
                                )

                # P1[(g,o)] = sum_{c,s} w3[o,c,s] * T[(g,c), s] via 9 accumulating
                # block-diagonal matmuls.
                p1 = psum_pool.tile([128, 1], fp32, tag="p1")
                for s in range(9):
                    nc.tensor.matmul(
                        p1,
                        lhsT=w3s[:, 128 * s : 128 * (s + 1)],
                        rhs=T[:, s : s + 1],
                        start=(s == 0),
                        stop=(s == 8),
                    )
                # All PSUM copy-outs happen on DVE so PSUM-slot WAR deps and
                # matmul rhs deps stay on a single proc (PE 1-wait limit).
                s1 = small.tile([128, 1], fp32, tag="s1")
                nc.vector.tensor_copy(out=s1, in_=p1)

                # y1 = relu(fc1 @ (P1/HW) + fc1 @ b3); 1/HW and fc1@b3 folded
                # host-side. relu+bias as one DVE tensor_scalar: (p2+b) max 0.
                p2 = psum_pool.tile([128, 1], fp32, tag="p2")
                nc.tensor.matmul(p2, lhsT=fcs[:, 0:128], rhs=s1, start=True, stop=True)
                s2 = small.tile([128, 1], fp32, tag="s2")
                nc.vector.tensor_scalar(
                    out=s2, in0=p2, scalar1=bv[:, 0:1], scalar2=0.0,
                    op0=ALU.add, op1=ALU.max,
                )

                # y2 = sigmoid(fc2 @ y1); weights = sigmoid(att + y2)
                p3 = psum_pool.tile([128, 1], fp32, tag="p3")
                nc.tensor.matmul(p3, lhsT=fcs[:, 128:256], rhs=s2, start=True, stop=True)
                s3p = small.tile([128, 1], fp32, tag="s3p")
                nc.vector.tensor_copy(out=s3p, in_=p3)
                s3 = small.tile([128, 1], fp32, tag="s3")
                nc.scalar.activation(out=s3, in_=s3p, func=AF.Sigmoid)
                wcol = small.tile([128, 1], fp32, tag="wcol")
                nc.scalar.activation(out=wcol, in_=s3, func=AF.Sigmoid, bias=bv[:, 1:2], scale=1.0)

                # out = x * weights (per-partition scalar broadcast), on DVE:
                # DVE already observed this tile's load-lane sem (the reduces
                # waited on it), and the wcol ACT dep is absorbed by a 1-elem
                # DVE copy, so the big multiply carries at most the one
                # store-WAR wait (PE/ACT/DVE instrs only support 1 sem wait).
                wcol_d = small.tile([128, 1], fp32, tag="wcol_d")
                nc.vector.tensor_copy(out=wcol_d, in_=wcol)
                # DVE emits explicit self-waits for producers within ~2
                # instructions (pipeline hazard window); a dependent chain of
                # 1-elem copies pushes wcol_d out of that window so the big
                # multiply's only sem wait is the store-WAR.
                dscr = small.tile([128, 3], fp32, tag="dscr")
                nc.vector.tensor_copy(out=dscr[:, 0:1], in_=wcol_d)
                nc.vector.tensor_copy(out=dscr[:, 1:2], in_=dscr[:, 0:1])
                nc.vector.tensor_copy(out=dscr[:, 2:3], in_=dscr[:, 1:2])
                nc.vector.tensor_scalar_mul(out=X, in0=X, scalar1=wcol_d)
                nc.sync.dma_start(out=out[128 * t : 128 * (t + 1), :], in_=X)

    return nc


def _get_bass():
    if "nc" not in _BASS_CACHE:
        _BASS_CACHE["nc"] = _build_bass()
    return _BASS_CACHE["nc"]


def _host_params(w3, b3, fc1, fc2, gn_b):
    """Build block-diagonal weights + bias vectors (tiny host-side prep)."""
    w3 = np.asarray(w3, np.float32)
    b3 = np.asarray(b3, np.float32)
    fc1 = np.asarray(fc1, np.float32)[:, :, 0, 0]
    fc2 = np.asarray(fc2, np.float32)[:, :, 0, 0]
    gn_b = np.asarray(gn_b, np.float64)

    e = np.exp(gn_b - gn_b.max())
    att = (e / e.sum()).astype(np.float32)

    w3bd = np.zeros((9, 128, 128), np.float32)
    fc1bd = np.zeros((128, 128), np.float32)
    fc2bd = np.zeros((128, 128), np.float32)
    for k in range(GPS):
        a = 16 * k
        fc1bd[a : a + 16, a : a + 16] = fc1.T / HW
        fc2bd[a : a + 16, a : a + 16] = fc2.T
        for s in range(9):
            si, sj = divmod(s, 3)
            w3bd[s, a : a + 16, a : a + 16] = w3[:, :, si, sj].T
    w3bd_dram = np.ascontiguousarray(w3bd.transpose(1, 0, 2).reshape(128, 9 * 128))
    fcbd = np.ascontiguousarray(np.concatenate([fc1bd, fc2bd], axis=1))
    hb1 = (fc1 @ b3).astype(np.float32)
    bvec = np.stack([np.tile(hb1, GPS), np.tile(att, GPS)], axis=1).astype(np.float32)
    return np.ascontiguousarray(
        np.concatenate([w3bd_dram, fcbd, bvec], axis=1)
    )  # [128, 1410]


def kernel(**inputs):
    from concourse.bass_utils import run_bass_kernel_spmd

    x = np.ascontiguousarray(np.asarray(inputs["inputs"], np.float32))
    assert x.shape == (NCORES, CH, H, W)
    params = _host_params(
        inputs["w3"], inputs["b3"], inputs["fc1_w"], inputs["fc2_w"], inputs["gn_b"]
    )

    nc = _get_bass()
    in_maps = [
        {"x": x[b].reshape(CH, HW), "params": params} for b in range(NCORES)
    ]
    res = run_bass_kernel_spmd(nc, in_maps, core_ids=list(range(NCORES)))
    return np.stack(
        [res.results[b]["out"].reshape(CH, H, W) for b in range(NCORES)]
    ).astype(np.float32)
